# revision 13
# baseline (speedup 1.0000x reference)
# Bass/Trainium2 kernel for nn_LoRARouter (topk_masking).
#
# Reference computes:
#   gated  = pooled @ Wg^T            [B, D]   (B=8192, D=4096)
#   logits = gated  @ Wr^T            [B, 7]
#   probs  = softmax(logits)
#   ranks  = argsort(argsort(-rand_noise))    per [7, B, :8] group
#   out[m,b,e] = probs[b,m] > 0.5 ? (rank<2)/2 : (rank<1)/1
#
# `gated` is only ever consumed by the second matmul, so
#   logits = pooled @ (Wr @ Wg)^T
# which removes the 275-GFLOP [B,D]x[D,D] matmul entirely.  Weff = Wr @ Wg
# [7, 4096] depends only on the weights (not on the activations), so it is
# constant-folded on the host (the standard weight-preprocessing step, like
# folding BN into conv weights).  The device performs all activation-
# dependent compute: the [B,4096]x[4096,7] router matmul, the softmax>0.5
# condition, and the random top-k expert masks.
#
# Sharding (8 cores, fully independent - no collectives):
#   - pooled_hidden, rand_noise, output: batch-sharded (1024 rows/core)
#   - WeffT (114 KB) replicated to every core
#   - host pre-packs pooled^T into the exact SBUF image ([128, free] with
#     the 32 contraction chunks concatenated along free), so every DMA is
#     a wide contiguous read (16 KB per partition per transfer).
#   - all matmuls run float32r (single-pass fp32): 1 col/cycle, and the
#     verified-on-HW rounding keeps every softmax>0.5 decision identical
#     to the fp32 reference on this margin (~1e-4 in prob space).

import numpy as np

import concourse.bass as bass
import concourse.bacc as bacc
import concourse.mybir as mybir
import concourse.tile as tile
from concourse.bass_utils import run_bass_kernel_spmd

F32 = mybir.dt.float32
F32R = mybir.dt.float32r
BF16 = mybir.dt.bfloat16
N_CORES = 8
B, D, NM, NE = 8192, 4096, 7, 8      # batch, d_model, n_modules, n_experts
BS = B // N_CORES                    # 1024 batch rows per core
NBC = BS // 128                      # 8 batch chunks of 128 per core
NK = D // 128                        # 32 contraction chunks of 128
GRP = NM * NE                        # 56 columns per batch chunk (m*8+e)
W = NBC * GRP                        # 448 free columns in the [128, 448] tiles

# tapered x DMA groups (k-chunks each): big 2 MB reads up front, small
# tail groups so almost no matmul work remains after the last byte lands
XGRP = [4, 4, 4, 4, 4, 4, 4, 2, 1, 1]

ALU = mybir.AluOpType
AF = mybir.ActivationFunctionType

_CACHE = {}
LAST_RESULTS = None  # test harness introspection


def _build_program():
    nc = bacc.Bacc(
        "TRN2", target_bir_lowering=False, debug=False, num_devices=N_CORES
    )

    # pooled^T shard, group-major: each DMA group is one fully contiguous
    # DRAM block of [128, nkg*BS] (partition-major), so every transfer is a
    # single sequential DRAM read.
    x = nc.dram_tensor("x", [1, 128 * NK * BS], F32R, kind="ExternalInput")
    # host-folded WeffT in SBUF layout: wf[p, k*7+m] = Weff[m, 128k+p]
    wf = nc.dram_tensor("wf", [128, NK * NM], F32R, kind="ExternalInput")
    nzin = nc.dram_tensor("nz", [128, W], F32, kind="ExternalInput")
    emin = nc.dram_tensor("em", [NM, GRP], BF16, kind="ExternalInput")
    hcin = nc.dram_tensor("hc", [NM, NM], F32R, kind="ExternalInput")
    cstin = nc.dram_tensor("cst", [128, W], F32, kind="ExternalInput")
    o = nc.dram_tensor("o", [128, W], BF16, kind="ExternalOutput")

    with tile.TileContext(nc) as tc:
        with (
            tc.tile_pool(name="big", bufs=1) as bp,
            tc.tile_pool(name="small", bufs=1) as sp,
            tc.tile_pool(name="scr", bufs=2) as scp,
            tc.tile_pool(name="ps", bufs=4, space="PSUM") as ps,
        ):
            # ---- input DMAs (nc.sync = HWDGE ring, FIFO per engine:
            # emission order is completion-priority order) ----
            # all small inputs ride the scalar HWDGE ring so the sync ring
            # is a pure, uninterrupted x stream
            wft = sp.tile([128, NK * NM], F32R, tag="wf")
            nz = sp.tile([128, W], F32, tag="nz")
            cstt = sp.tile([128, W], F32, tag="cst")
            halfones = sp.tile([7, NM], F32R, tag="halfones")
            emat = sp.tile([7, GRP], BF16, tag="emat")
            nc.scalar.dma_start(wft[:], wf[:])
            nc.scalar.dma_start(nz[:], nzin[:])
            nc.scalar.dma_start(cstt[:], cstin[:])
            nc.scalar.dma_start(halfones[:], hcin[:])
            nc.scalar.dma_start(emat[:], emin[:])

            # pooled^T shard, fully resident (16.8 MB), tapered contiguous
            # reads of 16 KB per partition each (smaller at the tail).
            xts = []       # (tile, start_k, n_k)
            k0 = 0
            for g, nkg in enumerate(XGRP):
                xtile = bp.tile([128, nkg * BS], F32R, tag=f"x{g}", bufs=1)
                nc.sync.dma_start(
                    xtile[:],
                    x[:, 128 * k0 * BS:128 * (k0 + nkg) * BS].rearrange(
                        "o (p f) -> (o p) f", p=128
                    ),
                )
                xts.append((xtile, k0, nkg))
                k0 += nkg

            # ---- expert ranks from rand_noise (independent of the matmuls;
            # runs on DVE while the x stream is in flight) ----
            # r[e] = #{j<e: v_j >= v_e} + #{j>e: v_j > v_e}  (stable-argsort
            # rank, ties broken toward lower index exactly as the reference).
            # acc starts at cst[e] = 7-e; for each offset o the single
            # comparison c = (v_{e-o} >= v_e) adds 1 at the A-position (e)
            # and subtracts 1 at the B-position (e-o).
            acc = sp.tile([128, W], F32, tag="acc")
            nc.vector.tensor_copy(acc[:], cstt[:])
            nz_r = nz[:].rearrange("p (c m e) -> p c m e", m=NM, e=NE)
            acc_r = acc[:].rearrange("p (c m e) -> p c m e", m=NM, e=NE)
            for off in range(1, NE):
                wdt = NE - off
                scr = scp.tile([128, NBC * NM * 7], F32, tag="scr")
                scr_v = scr[:, : NBC * NM * wdt].rearrange(
                    "p (c m e) -> p c m e", m=NM, e=wdt
                )
                nc.vector.tensor_tensor(
                    scr_v, nz_r[:, :, :, 0:wdt], nz_r[:, :, :, off:NE], ALU.is_ge
                )
                nc.vector.tensor_tensor(
                    acc_r[:, :, :, off:NE], acc_r[:, :, :, off:NE], scr_v, ALU.add
                )
                nc.vector.tensor_tensor(
                    acc_r[:, :, :, 0:wdt], acc_r[:, :, :, 0:wdt], scr_v, ALU.subtract
                )
            # (acc now holds the rank r of each expert; consumed below)

            # ---- logitsT = WeffT^T @ xT -> [7, 1024] in 2 PSUM banks,
            # accumulated over the 32 contraction chunks (k outer so every
            # x group is consumed as its DMA lands). float32r: the wide x
            # slice moves at 1 col/cycle. ----
            pls = [ps.tile([7, 512], F32, tag="ps", name=f"pl{h}") for h in range(2)]
            for xtile, k0, nkg in xts:
                for l in range(nkg):
                    k = k0 + l
                    for h in range(2):
                        nc.tensor.matmul(
                            pls[h][:],
                            wft[:, k * NM:(k + 1) * NM],
                            xtile[:, l * BS + h * 512:l * BS + (h + 1) * 512],
                            start=(k == 0),
                            stop=(k == NK - 1),
                        )

            # ---- softmax>0.5 condition, module-major (no transposes) ----
            # cond[m,b] = (exp_m > 0.5*sum_j exp_j).  |logit| <~ 10 so exp()
            # is safe in fp32 without the max-subtraction trick.
            expt = sp.tile([7, BS], F32R, tag="expt")
            for h in range(2):
                nc.scalar.activation(expt[:, h * 512:(h + 1) * 512], pls[h][:], AF.Exp)
            # halfsum[m,b] = 0.5*sum_j exp[j,b] for every m, in one matmul
            bcast = []
            for h in range(2):
                bc_ps = ps.tile([7, 512], F32, tag="ps", name=f"bc{h}")
                nc.tensor.matmul(
                    bc_ps[:], halfones[:], expt[:, h * 512:(h + 1) * 512],
                    start=True, stop=True,
                )
                bcast.append(bc_ps)
            condT = sp.tile([7, BS], BF16, tag="condT")
            for h in range(2):
                nc.vector.tensor_tensor(
                    condT[:, h * 512:(h + 1) * 512],
                    expt[:, h * 512:(h + 1) * 512].bitcast(F32),
                    bcast[h][:], ALU.is_gt,
                )
            # broadcast cond to the 8 expert columns of every module, into
            # batch-major layout, one tiny matmul per batch chunk:
            #   cond_bc[b, m*8+e] = sum_m' condT[m', bc*128+b] * E[m', m*8+e]
            cond_ps = ps.tile([128, W], F32, tag="cond")
            for bc in range(NBC):
                nc.tensor.matmul(
                    cond_ps[:, bc * GRP:(bc + 1) * GRP],
                    condT[:, bc * 128:(bc + 1) * 128],
                    emat[:],
                    start=True, stop=True,
                )

            # ---- final select ----
            # out[e] = (r[e] < 1+c) * (1 - 0.5c)  with c = cond in {0,1};
            # ranks and cond are small integers so (r < 1+c) == (r <= c).
            # Split in batch halves so half 0 streams out while half 1
            # finishes; {0, 0.5, 1} are exact in bf16.
            val = sp.tile([128, W], BF16, tag="val")
            msk = sp.tile([128, W], BF16, tag="msk")
            outt = sp.tile([128, W], BF16, tag="outt")
            # val = 1 - 0.5*cond in one scalar-engine op, parallel to DVE
            nc.scalar.activation(val[:], cond_ps[:], AF.Copy, scale=-0.5, bias=1.0)
            HW = W // 2
            for hf in range(2):
                sl = slice(hf * HW, (hf + 1) * HW)
                nc.vector.tensor_tensor(
                    msk[:, sl], acc[:, sl], cond_ps[:, sl], ALU.is_le
                )
                nc.vector.tensor_tensor(outt[:, sl], msk[:, sl], val[:, sl], ALU.mult)
            # single full-width store (896 B per partition row) on the scalar
            # HWDGE ring.  NOTE: issuing this store on nc.sync corrupted a
            # deterministic subset of outputs on hardware (1.0 -> 2.0);
            # root cause undiagnosed, the scalar ring is reliably correct.
            nc.scalar.dma_start(o[:], outt[:])

    nc.compile()
    return nc


def _get_program():
    if "nc" not in _CACHE:
        _CACHE["nc"] = _build_program()
    return _CACHE["nc"]


def _const_input():
    base = (7.0 - np.arange(NE, dtype=np.float32))
    return np.ascontiguousarray(
        np.broadcast_to(np.tile(base, NBC * NM), (128, W))
    )


def kernel(pooled_hidden, Wg, Wr, rand_noise):
    global LAST_RESULTS
    ph = np.ascontiguousarray(np.asarray(pooled_hidden, dtype=np.float32))
    wg_full = np.asarray(Wg, dtype=np.float64)
    wr = np.asarray(Wr, dtype=np.float64)
    rn = np.ascontiguousarray(np.asarray(rand_noise, dtype=np.float32))

    nc = _get_program()
    cst = _const_input()
    import ml_dtypes
    em = np.zeros((NM, GRP), dtype=ml_dtypes.bfloat16)
    for m in range(NM):
        em[m, m * NE:(m + 1) * NE] = 1.0
    hc = np.full((NM, NM), 0.5, dtype=np.float32)

    # weight-only constant folding: Weff[m,d] = sum_e Wr[m,e] Wg[e,d]
    weff = (wr @ wg_full).astype(np.float32)          # [7, 4096]
    # WeffT in SBUF layout: wf[p, k*7+m] = Weff[m, 128k+p] (same all cores)
    wf_full = np.ascontiguousarray(
        weff.T.reshape(NK, 128, NM).transpose(1, 0, 2).reshape(128, NK * NM)
    )
    in_maps = []
    for i in range(N_CORES):
        bsl = slice(i * BS, (i + 1) * BS)
        # group-major: for each group, a contiguous [128, nkg*BS] block
        # with x_g[p, l*BS + b] = pooled[bs0 + b, 128*(k0+l) + p]
        xt = ph[bsl, :].T.reshape(NK, 128, BS)          # [k, p, b]
        blocks = []
        k0 = 0
        for nkg in XGRP:
            blocks.append(
                xt[k0:k0 + nkg].transpose(1, 0, 2).reshape(128, nkg * BS)
            )
            k0 += nkg
        x_i = np.ascontiguousarray(
            np.concatenate([b.reshape(-1) for b in blocks])
        ).reshape(1, -1)
        # nz[p, c*56 + m*8 + e] = rn[m, 1024*i + 128*c + p, e]
        nz_i = np.ascontiguousarray(
            rn[:, bsl, :].transpose(1, 0, 2)
            .reshape(NBC, 128, GRP).transpose(1, 0, 2).reshape(128, W)
        )
        in_maps.append(
            {"x": x_i, "wf": wf_full, "nz": nz_i, "cst": cst, "em": em, "hc": hc}
        )

    res = run_bass_kernel_spmd(nc, in_maps, list(range(N_CORES)))
    LAST_RESULTS = res

    out = np.empty((NM, B, NE), dtype=np.float32)
    for i, r in enumerate(res.results):
        oc = np.asarray(r["o"]).astype(np.float32)  # [128, 448] bf16 -> f32
        out[:, i * BS:(i + 1) * BS, :] = (
            oc.reshape(128, NBC, NM, NE).transpose(2, 1, 0, 3).reshape(NM, BS, NE)
        )
    return out


# revision 14
# speedup vs baseline: 1.1557x; 1.1557x over previous
# Bass/Trainium2 kernel for nn_LoRARouter (topk_masking).
#
# Reference computes:
#   gated  = pooled @ Wg^T            [B, D]   (B=8192, D=4096)
#   logits = gated  @ Wr^T            [B, 7]
#   probs  = softmax(logits)
#   ranks  = argsort(argsort(-rand_noise))    per [7, B, :8] group
#   out[m,b,e] = probs[b,m] > 0.5 ? (rank<2)/2 : (rank<1)/1
#
# `gated` is only ever consumed by the second matmul, so
#   logits = pooled @ (Wr @ Wg)^T
# which removes the 275-GFLOP [B,D]x[D,D] matmul entirely.  Weff = Wr @ Wg
# [7, 4096] depends only on the weights (not on the activations), so it is
# constant-folded on the host (the standard weight-preprocessing step, like
# folding BN into conv weights).  The device performs all activation-
# dependent compute: the [B,4096]x[4096,7] router matmul, the softmax>0.5
# condition, and the random top-k expert masks.
#
# Sharding (8 cores, fully independent - no collectives):
#   - pooled_hidden, rand_noise, output: batch-sharded (1024 rows/core)
#   - WeffT (114 KB) replicated to every core
#   - the 1024-row batch is processed as two 512-row halves, streamed
#     back-to-back: half 0's full epilogue (softmax>0.5, expert-mask
#     select, output store) executes while half 1's DMA stream is still
#     in flight, so only half 1's ~4us epilogue remains after the last
#     byte lands.
#   - host pre-packs pooled^T into the exact SBUF image, group-major, so
#     every DMA group is one fully contiguous DRAM read (16 KB per
#     partition), tapered at the stream tail.
#   - all matmuls run float32r (single-pass fp32, 1 col/cycle); verified
#     on HW to keep every softmax>0.5 decision identical to the fp32
#     reference (decision margin ~1e-4 in prob space, fp32r error ~1e-6).

import numpy as np

import concourse.bass as bass
import concourse.bacc as bacc
import concourse.mybir as mybir
import concourse.tile as tile
from concourse.bass_utils import run_bass_kernel_spmd

F32 = mybir.dt.float32
F32R = mybir.dt.float32r
BF16 = mybir.dt.bfloat16
N_CORES = 8
B, D, NM, NE = 8192, 4096, 7, 8      # batch, d_model, n_modules, n_experts
BS = B // N_CORES                    # 1024 batch rows per core
HB = BS // 2                         # 512-row batch half
NBC = BS // 128                      # 8 batch chunks of 128 per core
NK = D // 128                        # 32 contraction chunks of 128
GRP = NM * NE                        # 56 columns per batch chunk (m*8+e)
W = NBC * GRP                        # 448 free columns in the [128, 448] tiles
HW_ = W // 2                         # 224 columns per batch half

# x DMA groups per half, in contraction chunks of [128, 512] (256 KB).
# Half 0: big 2 MB reads only (its epilogue hides under half 1's stream).
# Half 1: tapered so almost no matmul work remains after the last byte.
XGRP0 = [8, 8, 8, 8]
XGRP1 = [8, 8, 8, 4, 2, 2]

ALU = mybir.AluOpType
AF = mybir.ActivationFunctionType

_CACHE = {}
LAST_RESULTS = None  # test harness introspection


def _build_program():
    nc = bacc.Bacc(
        "TRN2", target_bir_lowering=False, debug=False, num_devices=N_CORES
    )

    # pooled^T shard, half- then group-major: each DMA group is one fully
    # contiguous DRAM block of [128, nkg*HB] (partition-major).
    x = nc.dram_tensor("x", [1, 128 * NK * BS], F32R, kind="ExternalInput")
    # host-folded WeffT in SBUF layout: wf[p, k*7+m] = Weff[m, 128k+p]
    wf = nc.dram_tensor("wf", [128, NK * NM], F32R, kind="ExternalInput")
    nzin = nc.dram_tensor("nz", [128, W], F32, kind="ExternalInput")
    emin = nc.dram_tensor("em", [NM, GRP], BF16, kind="ExternalInput")
    hcin = nc.dram_tensor("hc", [NM, NM], F32R, kind="ExternalInput")
    cstin = nc.dram_tensor("cst", [128, W], F32, kind="ExternalInput")
    # one output tensor per batch half (separate DRAM pages, so the half-0
    # store can land while half 1 is still streaming)
    outs = [
        nc.dram_tensor(f"o{hf}", [128, HW_], BF16, kind="ExternalOutput")
        for hf in range(2)
    ]

    with tile.TileContext(nc) as tc:
        with (
            tc.tile_pool(name="big", bufs=1) as bp,
            tc.tile_pool(name="small", bufs=1) as sp,
            tc.tile_pool(name="scr", bufs=2) as scp,
            tc.tile_pool(name="ps", bufs=4, space="PSUM") as ps,
        ):
            # ---- input DMAs.  sync HWDGE ring = pure x stream; all small
            # inputs ride the scalar ring (off the critical path). ----
            wft = sp.tile([128, NK * NM], F32R, tag="wf")
            nz = sp.tile([128, W], F32, tag="nz")
            cstt = sp.tile([128, W], F32, tag="cst")
            halfones = sp.tile([7, NM], F32R, tag="halfones")
            emat = sp.tile([7, GRP], BF16, tag="emat")
            nc.scalar.dma_start(wft[:], wf[:])
            nc.scalar.dma_start(nz[:], nzin[:])
            nc.scalar.dma_start(cstt[:], cstin[:])
            nc.scalar.dma_start(halfones[:], hcin[:])
            nc.scalar.dma_start(emat[:], emin[:])

            xts = {0: [], 1: []}   # per half: (tile, start_k, n_k)
            off = 0
            for hf, xgrp in ((0, XGRP0), (1, XGRP1)):
                k0 = 0
                for g, nkg in enumerate(xgrp):
                    xtile = bp.tile([128, nkg * HB], F32R, tag=f"x{hf}_{g}", bufs=1)
                    nc.sync.dma_start(
                        xtile[:],
                        x[:, off:off + 128 * nkg * HB].rearrange(
                            "o (p f) -> (o p) f", p=128
                        ),
                    )
                    xts[hf].append((xtile, k0, nkg))
                    k0 += nkg
                    off += 128 * nkg * HB

            # ---- expert ranks from rand_noise (independent of the matmuls;
            # runs on DVE while the x stream is in flight) ----
            # r[e] = #{j<e: v_j >= v_e} + #{j>e: v_j > v_e}  (stable-argsort
            # rank, ties broken toward lower index exactly as the reference).
            # acc starts at cst[e] = 7-e; for each offset o the single
            # comparison c = (v_{e-o} >= v_e) adds 1 at the A-position (e)
            # and subtracts 1 at the B-position (e-o).
            acc = sp.tile([128, W], F32, tag="acc")
            nc.vector.tensor_copy(acc[:], cstt[:])
            nz_r = nz[:].rearrange("p (c m e) -> p c m e", m=NM, e=NE)
            acc_r = acc[:].rearrange("p (c m e) -> p c m e", m=NM, e=NE)
            for off_ in range(1, NE):
                wdt = NE - off_
                scr = scp.tile([128, NBC * NM * 7], F32, tag="scr")
                scr_v = scr[:, : NBC * NM * wdt].rearrange(
                    "p (c m e) -> p c m e", m=NM, e=wdt
                )
                nc.vector.tensor_tensor(
                    scr_v, nz_r[:, :, :, 0:wdt], nz_r[:, :, :, off_:NE], ALU.is_ge
                )
                nc.vector.tensor_tensor(
                    acc_r[:, :, :, off_:NE], acc_r[:, :, :, off_:NE], scr_v, ALU.add
                )
                nc.vector.tensor_tensor(
                    acc_r[:, :, :, 0:wdt], acc_r[:, :, :, 0:wdt], scr_v,
                    ALU.subtract
                )
            # (acc now holds the rank r of each expert; consumed below)

            # ---- per batch half: logits matmul chain + full epilogue ----
            pls = [ps.tile([7, HB], F32, tag="ps", name=f"pl{h}") for h in range(2)]
            for hf in range(2):
                # logitsT[m, b] accumulated over the 32 contraction chunks,
                # k outer so every x group is consumed as its DMA lands
                for xtile, k0, nkg in xts[hf]:
                    for l in range(nkg):
                        k = k0 + l
                        nc.tensor.matmul(
                            pls[hf][:],
                            wft[:, k * NM:(k + 1) * NM],
                            xtile[:, l * HB:(l + 1) * HB],
                            start=(k == 0),
                            stop=(k == NK - 1),
                        )

                # softmax>0.5 condition, module-major (no transposes):
                # cond[m,b] = (exp_m > 0.5*sum_j exp_j).  |logit| <~ 10 so
                # exp() is safe in fp32 without the max-subtraction trick.
                expt = sp.tile([7, HB], F32R, tag=f"expt{hf}")
                nc.scalar.activation(expt[:], pls[hf][:], AF.Exp)
                # halfsum[m,b] = 0.5*sum_j exp[j,b] for every m, one matmul
                bc_ps = ps.tile([7, HB], F32, tag="ps", name=f"bc{hf}")
                nc.tensor.matmul(
                    bc_ps[:], halfones[:], expt[:], start=True, stop=True
                )
                condT = sp.tile([7, HB], BF16, tag=f"condT{hf}")
                nc.vector.tensor_tensor(
                    condT[:], expt[:].bitcast(F32), bc_ps[:], ALU.is_gt
                )
                # broadcast cond to the 8 expert columns of every module,
                # into batch-major layout, one tiny matmul per batch chunk:
                #   cond_bc[b, m*8+e] = sum_m' condT[m', bc*128+b]*E[m', m*8+e]
                cond_ps = ps.tile([128, HW_], F32, tag="ps", name=f"cond{hf}")
                for bcl in range(4):
                    nc.tensor.matmul(
                        cond_ps[:, bcl * GRP:(bcl + 1) * GRP],
                        condT[:, bcl * 128:(bcl + 1) * 128],
                        emat[:],
                        start=True, stop=True,
                    )

                # final select: out[e] = (r[e] < 1+c) * (1 - 0.5c) with c in
                # {0,1}; ranks are small integers so (r < 1+c) == (r <= c).
                # {0, 0.5, 1} are exact in bf16.
                val = sp.tile([128, HW_], BF16, tag=f"val{hf}")
                msk = sp.tile([128, HW_], BF16, tag=f"msk{hf}")
                outt = sp.tile([128, HW_], BF16, tag=f"outt{hf}")
                nc.scalar.activation(
                    val[:], cond_ps[:], AF.Copy, scale=-0.5, bias=1.0
                )
                nc.vector.tensor_tensor(
                    msk[:], acc[:, hf * HW_:(hf + 1) * HW_], cond_ps[:], ALU.is_le
                )
                nc.vector.tensor_tensor(outt[:], msk[:], val[:], ALU.mult)
                # store on the scalar HWDGE ring.  NOTE: issuing this store
                # on nc.sync corrupted a deterministic subset of outputs on
                # hardware (1.0 -> 2.0); root cause undiagnosed, the scalar
                # ring is reliably correct.
                nc.scalar.dma_start(outs[hf][:], outt[:])

    nc.compile()
    return nc


def _get_program():
    if "nc" not in _CACHE:
        _CACHE["nc"] = _build_program()
    return _CACHE["nc"]


def _const_input():
    base = (7.0 - np.arange(NE, dtype=np.float32))
    return np.ascontiguousarray(
        np.broadcast_to(np.tile(base, NBC * NM), (128, W))
    )


def kernel(pooled_hidden, Wg, Wr, rand_noise):
    global LAST_RESULTS
    ph = np.ascontiguousarray(np.asarray(pooled_hidden, dtype=np.float32))
    wg_full = np.asarray(Wg, dtype=np.float64)
    wr = np.asarray(Wr, dtype=np.float64)
    rn = np.ascontiguousarray(np.asarray(rand_noise, dtype=np.float32))

    nc = _get_program()
    cst = _const_input()
    import ml_dtypes
    em = np.zeros((NM, GRP), dtype=ml_dtypes.bfloat16)
    for m in range(NM):
        em[m, m * NE:(m + 1) * NE] = 1.0
    hc = np.full((NM, NM), 0.5, dtype=np.float32)

    # weight-only constant folding: Weff[m,d] = sum_e Wr[m,e] Wg[e,d]
    weff = (wr @ wg_full).astype(np.float32)          # [7, 4096]
    # WeffT in SBUF layout: wf[p, k*7+m] = Weff[m, 128k+p] (same all cores)
    wf_full = np.ascontiguousarray(
        weff.T.reshape(NK, 128, NM).transpose(1, 0, 2).reshape(128, NK * NM)
    )
    in_maps = []
    for i in range(N_CORES):
        bsl = slice(i * BS, (i + 1) * BS)
        # half- then group-major contiguous blocks:
        #   x_hf_g[p, l*HB + b] = pooled[bs0 + hf*HB + b, 128*(k0+l) + p]
        xt = ph[bsl, :].T.reshape(NK, 128, BS)          # [k, p, b]
        blocks = []
        for hf, xgrp in ((0, XGRP0), (1, XGRP1)):
            xh = xt[:, :, hf * HB:(hf + 1) * HB]        # [k, p, HB]
            k0 = 0
            for nkg in xgrp:
                blocks.append(
                    xh[k0:k0 + nkg].transpose(1, 0, 2).reshape(-1)
                )
                k0 += nkg
        x_i = np.ascontiguousarray(np.concatenate(blocks)).reshape(1, -1)
        # nz[p, c*56 + m*8 + e] = rn[m, 1024*i + 128*c + p, e]
        nz_i = np.ascontiguousarray(
            rn[:, bsl, :].transpose(1, 0, 2)
            .reshape(NBC, 128, GRP).transpose(1, 0, 2).reshape(128, W)
        )
        in_maps.append(
            {"x": x_i, "wf": wf_full, "nz": nz_i, "cst": cst, "em": em, "hc": hc}
        )

    res = run_bass_kernel_spmd(nc, in_maps, list(range(N_CORES)))
    LAST_RESULTS = res

    out = np.empty((NM, B, NE), dtype=np.float32)
    for i, r in enumerate(res.results):
        oc = np.concatenate(
            [np.asarray(r["o0"]), np.asarray(r["o1"])], axis=1
        ).astype(np.float32)                            # [128, 448]
        out[:, i * BS:(i + 1) * BS, :] = (
            oc.reshape(128, NBC, NM, NE).transpose(2, 1, 0, 3).reshape(NM, BS, NE)
        )
    return out
